# revision 10
# baseline (speedup 1.0000x reference)
"""Trainium2 Bass kernel for nn_CausalEncoder (GNN message passing MLP).

Math (reference):
    send = X @ A.T ; recv = X @ A
    h  = relu(concat([send, recv]) @ W1 + b1)
    He = relu(h @ W2 + b2)
    Z  = relu(concat([X, He]) @ W3 + b3)

Layer 1 collapses exactly: concat([send,recv]) @ W1 = X @ (A.T@W1[:10] + A@W1[10:]) =: X @ M1.
So per row (d=10): three chained 10->10 matmuls with relu, pure memory-bound.

Strategy (v2): all layout work happens on the HOST; the device only does
matmuls, relu passes and contiguous DMA.

  - Host rounds X to bf16 and packs it feature-major: partitions 0..119 hold
    12 row-slots x 10 features, columns are row-groups. Per core the input is
    a dense [120, C_DEV] bf16 tile; no on-chip transposes, pads, or strided
    access patterns.
  - Partition 120 is a ones-lane (memset once per buffer); all biases ride in
    the weight blocks: each 121x121 block = [[W, 0], [b, 1]], padded to
    128x128 so K=M=128.
  - Per 1024-column tile: load -> MM1 -> relu1(ACT) -> MM2 -> relu2(DVE) ->
    MM3a+MM3b accumulate -> relu3 (split ACT/DVE) -> store. All relus are
    pure max (PSUM fp32 -> SBUF bf16).
  - Loads issue on the SP HWDGE ring, stores on the GPSIMD SWDGE ring, so
    neither ACT nor the load ring queues behind compute-gated stores.
  - Host unpacks the bf16 [120, C_DEV] result back to f32 [B, 10].
"""

import numpy as np
import ml_dtypes

BF = ml_dtypes.bfloat16

B_TOTAL = 4_000_000
D = 10
N_CORES = 8
ROWS_PER_CORE = B_TOTAL // N_CORES
SLOTS = 12                     # row-slots per column
PD = SLOTS * D                 # 120 data partitions
ONES_P = PD                    # ones-lane partition
C_TILE = 1024                  # columns per compute tile
N_TILES = 41
C_DEV = N_TILES * C_TILE       # 41984 columns per core
R_CAP = C_DEV * SLOTS          # 503808 row capacity per core
XBUFS = 3                      # xin pool depth (memset-once count must match)


# ---------------------------------------------------------------------------
# Workarounds for this walrus build: it rejects >1 sem-wait per instruction
# on some opcodes. Split the Tile tail drain, and post-process every
# instruction, moving excess waits onto preceding same-engine NoOps.
# ---------------------------------------------------------------------------

def _apply_drain_patch():
    import concourse.tile as tile_mod
    import concourse.mybir as mybir
    from concourse.vector_clock import ScopedClock

    if getattr(tile_mod.TileContext, "_drain_patched", False):
        return

    def _patched_drain_and_barrier(self, tick_clock, wait_clock):
        nc = self.nc
        drain_inst = nc.sync.drain()
        wait_clock.add_sem_waits(
            drain_inst.ins, ScopedClock({None: tick_clock.global_clock})
        )
        si = drain_inst.ins.sync_info
        waits = list(si.on_wait or []) if si is not None else []
        if len(waits) > 1:
            si.on_wait = waits[:1]
            rest = waits[1:]
            while rest:
                d2 = nc.sync.drain()
                si2 = d2.ins.sync_info
                if si2 is None:
                    si2 = mybir.SyncInfo(on_wait=[], on_update=[])
                    d2.ins.sync_info = si2
                si2.on_wait = rest[:1]
                rest = rest[1:]

        nc.all_engine_barrier()
        assert self.sems is not None
        popped = nc._tile_sem_poison_stack.pop()
        assert popped is self._sem_poison
        nc.clear_and_free_semaphores(list(self.sems.allocated().values()))
        nc.all_engine_barrier()

    tile_mod.TileContext._drain_and_barrier = _patched_drain_and_barrier
    tile_mod.TileContext._drain_patched = True


def _apply_verifier_patch():
    """Drop the birverifier walrus pass (kept from the previous kernel; it
    rejects some numerically-fine dtype plumbing)."""
    import concourse.bass_utils as bu

    if getattr(bu, "_verifier_patched", False):
        return
    orig = bu.run_command

    def patched_run_command(argv, **kwargs):
        argv = [
            a.replace("birverifier,", "") if isinstance(a, str) else a
            for a in argv
        ]
        return orig(argv, **kwargs)

    bu.run_command = patched_run_command
    bu._verifier_patched = True


def _split_sync_waits(nc, limit=1):
    """Cap per-instruction sem waits for this walrus build."""
    import concourse.mybir as mybir

    uid = 0
    for fn in nc.m.functions:
        for bb in fn.blocks:
            new_insts = []
            for inst in bb.instructions:
                kind = type(inst).__name__
                if kind in ("InstStreamTranspose", "InstTensorScalarPtr",
                            "InstTensorTensor", "InstTensorCopy") and str(
                    inst.engine
                ).endswith("DVE"):
                    lim = limit
                else:
                    lim = 1
                si = inst.sync_info
                waits = list(si.on_wait) if si is not None and si.on_wait else []
                if len(waits) > lim:
                    keep = waits[-lim:]
                    excess = waits[:-lim]
                    for w in excess:
                        uid += 1
                        new_insts.append(
                            mybir.InstNoOp(
                                name=f"I-syncsplit-{uid}",
                                engine=inst.engine,
                                sync_info=mybir.SyncInfo(on_wait=[w], on_update=[]),
                            )
                        )
                    si.on_wait = keep
                new_insts.append(inst)
            bb.instructions[:] = new_insts


# ---------------------------------------------------------------------------
# Host-side weight preprocessing
# ---------------------------------------------------------------------------

def _block_weights(W, bias=None, ones=False):
    """[10,10] weight + optional bias row -> [128,128] bf16: 12 diagonal
    10x10 blocks, bias broadcast from the ones-lane row, optional ones
    passthrough at [120,120]."""
    blk = np.zeros((128, 128), np.float32)
    for g in range(SLOTS):
        blk[D * g:D * g + D, D * g:D * g + D] = W
        if bias is not None:
            blk[ONES_P, D * g:D * g + D] = bias
    if ones:
        blk[ONES_P, ONES_P] = 1.0
    return blk.astype(BF)


def _prep_consts(A, W1, b1, W2, b2, W3, b3):
    A64 = np.asarray(A, np.float64)
    W164 = np.asarray(W1, np.float64)
    M1 = (A64.T @ W164[:D] + A64 @ W164[D:]).astype(np.float32)
    return {
        "BD1": _block_weights(M1, np.asarray(b1, np.float32), ones=True),
        "BD2": _block_weights(np.asarray(W2, np.float32),
                              np.asarray(b2, np.float32), ones=True),
        "BD3a": _block_weights(np.asarray(W3[:D], np.float32),
                               np.asarray(b3, np.float32), ones=False),
        "BD3b": _block_weights(np.asarray(W3[D:], np.float32), ones=False),
    }


# ---------------------------------------------------------------------------
# Bass program
# ---------------------------------------------------------------------------

def _build_program(split_waits=True, n_tiles=None):
    import concourse.bass as bass
    import concourse.mybir as mybir
    from concourse.tile import TileContext

    f32 = mybir.dt.float32
    bf16 = mybir.dt.bfloat16
    Relu = mybir.ActivationFunctionType.Relu
    H = C_TILE // 2  # 512, one PSUM bank per matmul

    nc = bass.Bass("TRN2", target_bir_lowering=False, debug=False)
    Xc = nc.dram_tensor("Xc", [PD, C_DEV], bf16, kind="ExternalInput")
    Zc = nc.dram_tensor("Zc", [PD, C_DEV], bf16, kind="ExternalOutput")
    CP = nc.dram_tensor("CPAD", [128 - PD, C_TILE], bf16, kind="ExternalInput")
    dws = {n: nc.dram_tensor(n, [128, 128], bf16, kind="ExternalInput")
           for n in ("BD1", "BD2", "BD3a", "BD3b")}

    xa, za = Xc.ap(), Zc.ap()
    T = N_TILES if n_tiles is None else n_tiles

    # 2-tile weight groups: each of the 4 weight matrices is loaded once per
    # group (ldw-opt dedups the repeat LDWEIGHTS), so same-weight matmuls
    # pipeline back-to-back. PSUM: h and z share one pool (h is freed by
    # relu1 before the z matmuls run) -> (2+2)*2 banks = 8, exactly PSUM.
    groups = [(2 * g, 2 * g + 1) for g in range(T // 2)]
    if T % 2:
        groups.append((T - 1,))

    with TileContext(nc) as tc:
        with (
            tc.tile_pool(name="consts", bufs=1) as cpool,
            tc.tile_pool(name="xin", bufs=5) as xpool,
            tc.tile_pool(name="mid", bufs=3) as midpool,
            tc.tile_pool(name="zout", bufs=4) as zpool,
            tc.tile_pool(name="pshz", bufs=2, space="PSUM") as pshz,
            tc.tile_pool(name="pshe", bufs=2, space="PSUM") as pshe,
        ):
            sw = {}
            for n in ("BD1", "BD2", "BD3a", "BD3b"):
                t = cpool.tile([128, 128], bf16, tag=n)
                nc.sync.dma_start(out=t, in_=dws[n].ap())
                sw[n] = t

            # ones-lane / zero-pad partitions: written once per xin buffer,
            # loads only ever touch [0:PD]
            for _ in range(5):
                t = xpool.tile([128, C_TILE], bf16, tag="xin")
                nc.sync.dma_start(out=t[PD:128, :], in_=CP.ap())

            st = {}

            def stage_load(it):
                xin = xpool.tile([128, C_TILE], bf16, tag="xin")
                nc.sync.dma_start(
                    out=xin[0:PD, :], in_=xa[:, it * C_TILE:(it + 1) * C_TILE]
                )
                st[it] = {"xin": xin}

            def stage_compute(tiles):
                hps, hsb, heps, hesb, zps, zt = {}, {}, {}, {}, {}, {}
                for it in tiles:
                    hps[it] = pshz.tile([128, C_TILE], f32, tag="hz", name="hps")
                    for j in (0, 1):
                        nc.tensor.matmul(
                            hps[it][:, H * j:H * (j + 1)], sw["BD1"],
                            st[it]["xin"][:, H * j:H * (j + 1)],
                            start=True, stop=True,
                        )
                for it in tiles:
                    hsb[it] = midpool.tile([128, C_TILE], bf16, tag="hsb", name="hsb")
                    nc.scalar.activation(hsb[it], hps[it], Relu)
                for it in tiles:
                    heps[it] = pshe.tile([128, C_TILE], f32, tag="he", name="heps")
                    for j in (0, 1):
                        nc.tensor.matmul(
                            heps[it][:, H * j:H * (j + 1)], sw["BD2"],
                            hsb[it][:, H * j:H * (j + 1)],
                            start=True, stop=True,
                        )
                for it in tiles:
                    hesb[it] = midpool.tile([128, C_TILE], bf16, tag="hesb", name="hesb")
                    nc.vector.tensor_scalar_max(hesb[it], heps[it], 0.0)
                for it in tiles:
                    zps[it] = pshz.tile([128, C_TILE], f32, tag="hz", name="zps")
                    xin = st[it].pop("xin")
                    for j in (0, 1):
                        nc.tensor.matmul(
                            zps[it][:, H * j:H * (j + 1)], sw["BD3a"],
                            xin[:, H * j:H * (j + 1)], start=True, stop=False,
                        )
                for it in tiles:
                    for j in (0, 1):
                        nc.tensor.matmul(
                            zps[it][:, H * j:H * (j + 1)], sw["BD3b"],
                            hesb[it][:, H * j:H * (j + 1)],
                            start=False, stop=True,
                        )
                for it in tiles:
                    zt[it] = zpool.tile([128, C_TILE], bf16, tag="zt", name="zt")
                    nc.scalar.activation(zt[it][:, 0:H], zps[it][:, 0:H], Relu)
                    nc.vector.tensor_scalar_max(
                        zt[it][:, H:], zps[it][:, H:], 0.0
                    )
                    st[it]["zt"] = zt[it]

            def stage_store(tiles):
                for it in tiles:
                    zt = st.pop(it)["zt"]
                    nc.gpsimd.dma_start(
                        out=za[:, it * C_TILE:(it + 1) * C_TILE],
                        in_=zt[0:PD, :],
                    )

            # software-pipelined emission over groups: loads lead by one
            # group, stores trail by one
            NG = len(groups)
            for step in range(NG + 2):
                if step < NG:
                    for it in groups[step]:
                        stage_load(it)
                if 0 <= step - 1 < NG:
                    stage_compute(groups[step - 1])
                if 0 <= step - 2 < NG:
                    stage_store(groups[step - 2])

    if split_waits:
        _split_sync_waits(nc, limit=1)
    return nc


_CACHED = {}


# ---------------------------------------------------------------------------
# Host-side pack / unpack
# ---------------------------------------------------------------------------

def _pack_inputs(X):
    """[B,10] f32 -> per-core [120, C_DEV] bf16, feature-major dense."""
    Xb = np.asarray(X, np.float32).astype(BF)
    Xp = np.zeros((N_CORES, R_CAP, D), BF)
    Xp[:, :ROWS_PER_CORE] = Xb.reshape(N_CORES, ROWS_PER_CORE, D)
    # [cores, C, slots, D] -> [cores, slots, D, C]
    Xt = Xp.reshape(N_CORES, C_DEV, SLOTS, D).transpose(0, 2, 3, 1)
    return [np.ascontiguousarray(Xt[c]).reshape(PD, C_DEV) for c in range(N_CORES)]


def _unpack_outputs(Zs):
    """per-core [120, C_DEV] bf16 -> [B,10] f32."""
    Z = np.stack(Zs).reshape(N_CORES, SLOTS, D, C_DEV)
    Z = Z.transpose(0, 3, 1, 2).reshape(N_CORES, R_CAP, D)[:, :ROWS_PER_CORE]
    return np.ascontiguousarray(Z.reshape(B_TOTAL, D)).astype(np.float32)


def kernel(X, A, W1, b1, W2, b2, W3, b3):
    _apply_drain_patch()
    _apply_verifier_patch()
    from concourse.bass_utils import run_bass_kernel_spmd

    consts = _prep_consts(A, W1, b1, W2, b2, W3, b3)

    if "nc" not in _CACHED:
        _CACHED["nc"] = _build_program()
    nc = _CACHED["nc"]

    cpad = np.zeros((128 - PD, C_TILE), BF)
    cpad[0] = 1.0  # ones-lane at partition PD
    xcores = _pack_inputs(X)
    in_maps = []
    for c in range(N_CORES):
        m = {"Xc": xcores[c], "CPAD": cpad}
        m.update(consts)
        in_maps.append(m)

    res = run_bass_kernel_spmd(nc, in_maps, core_ids=list(range(N_CORES)))
    _CACHED["last_results"] = res
    return _unpack_outputs([res.results[c]["Zc"] for c in range(N_CORES)])


# revision 11
# speedup vs baseline: 1.5156x; 1.5156x over previous
"""Trainium2 Bass kernel for nn_CausalEncoder (GNN message passing MLP).

Math (reference):
    send = X @ A.T ; recv = X @ A
    h  = relu(concat([send, recv]) @ W1 + b1)
    He = relu(h @ W2 + b2)
    Z  = relu(concat([X, He]) @ W3 + b3)

Layer 1 collapses exactly: concat([send,recv]) @ W1 = X @ (A.T@W1[:10] + A@W1[10:]) =: X @ M1.
So per row (d=10): three chained 10->10 matmuls with relu, pure memory-bound.

Strategy (v2): all layout work happens on the HOST; the device only does
matmuls, relu passes and contiguous DMA.

  - Host rounds X to bf16 and packs it feature-major: partitions 0..119 hold
    12 row-slots x 10 features, columns are row-groups. Per core the input is
    a dense [120, C_DEV] bf16 tile; no on-chip transposes, pads, or strided
    access patterns.
  - Partition 120 is a ones-lane (memset once per buffer); all biases ride in
    the weight blocks: each 121x121 block = [[W, 0], [b, 1]], padded to
    128x128 so K=M=128.
  - Per 1024-column tile: load -> MM1 -> relu1(ACT) -> MM2 -> relu2(DVE) ->
    MM3a+MM3b accumulate -> relu3 (split ACT/DVE) -> store. All relus are
    pure max (PSUM fp32 -> SBUF bf16).
  - Loads issue on the SP HWDGE ring, stores on the GPSIMD SWDGE ring, so
    neither ACT nor the load ring queues behind compute-gated stores.
  - Host unpacks the bf16 [120, C_DEV] result back to f32 [B, 10].
"""

import numpy as np
import ml_dtypes

BF = ml_dtypes.bfloat16

B_TOTAL = 4_000_000
D = 10
N_CORES = 8
ROWS_PER_CORE = B_TOTAL // N_CORES
SLOTS = 12                     # row-slots per column
PD = SLOTS * D                 # 120 data partitions
ONES_P = PD                    # ones-lane partition
C_TILE = 1024                  # columns per compute tile
N_TILES = 41
C_DEV = N_TILES * C_TILE       # 41984 columns per core
R_CAP = C_DEV * SLOTS          # 503808 row capacity per core
XBUFS = 3                      # xin pool depth (memset-once count must match)


# ---------------------------------------------------------------------------
# Workarounds for this walrus build: it rejects >1 sem-wait per instruction
# on some opcodes. Split the Tile tail drain, and post-process every
# instruction, moving excess waits onto preceding same-engine NoOps.
# ---------------------------------------------------------------------------

def _apply_drain_patch():
    import concourse.tile as tile_mod
    import concourse.mybir as mybir
    from concourse.vector_clock import ScopedClock

    if getattr(tile_mod.TileContext, "_drain_patched", False):
        return

    def _patched_drain_and_barrier(self, tick_clock, wait_clock):
        nc = self.nc
        drain_inst = nc.sync.drain()
        wait_clock.add_sem_waits(
            drain_inst.ins, ScopedClock({None: tick_clock.global_clock})
        )
        si = drain_inst.ins.sync_info
        waits = list(si.on_wait or []) if si is not None else []
        if len(waits) > 1:
            si.on_wait = waits[:1]
            rest = waits[1:]
            while rest:
                d2 = nc.sync.drain()
                si2 = d2.ins.sync_info
                if si2 is None:
                    si2 = mybir.SyncInfo(on_wait=[], on_update=[])
                    d2.ins.sync_info = si2
                si2.on_wait = rest[:1]
                rest = rest[1:]

        nc.all_engine_barrier()
        assert self.sems is not None
        popped = nc._tile_sem_poison_stack.pop()
        assert popped is self._sem_poison
        nc.clear_and_free_semaphores(list(self.sems.allocated().values()))
        nc.all_engine_barrier()

    tile_mod.TileContext._drain_and_barrier = _patched_drain_and_barrier
    tile_mod.TileContext._drain_patched = True


def _apply_verifier_patch():
    """Drop the birverifier walrus pass (kept from the previous kernel; it
    rejects some numerically-fine dtype plumbing)."""
    import concourse.bass_utils as bu

    if getattr(bu, "_verifier_patched", False):
        return
    orig = bu.run_command

    def patched_run_command(argv, **kwargs):
        argv = [
            a.replace("birverifier,", "") if isinstance(a, str) else a
            for a in argv
        ]
        return orig(argv, **kwargs)

    bu.run_command = patched_run_command
    bu._verifier_patched = True


def _split_sync_waits(nc, limit=1):
    """Cap per-instruction sem waits for this walrus build."""
    import concourse.mybir as mybir

    uid = 0
    for fn in nc.m.functions:
        for bb in fn.blocks:
            new_insts = []
            for inst in bb.instructions:
                kind = type(inst).__name__
                if kind in ("InstStreamTranspose", "InstTensorScalarPtr",
                            "InstTensorTensor", "InstTensorCopy") and str(
                    inst.engine
                ).endswith("DVE"):
                    lim = limit
                else:
                    lim = 1
                si = inst.sync_info
                waits = list(si.on_wait) if si is not None and si.on_wait else []
                if len(waits) > lim:
                    keep = waits[-lim:]
                    excess = waits[:-lim]
                    for w in excess:
                        uid += 1
                        new_insts.append(
                            mybir.InstNoOp(
                                name=f"I-syncsplit-{uid}",
                                engine=inst.engine,
                                sync_info=mybir.SyncInfo(on_wait=[w], on_update=[]),
                            )
                        )
                    si.on_wait = keep
                new_insts.append(inst)
            bb.instructions[:] = new_insts


# ---------------------------------------------------------------------------
# Host-side weight preprocessing
# ---------------------------------------------------------------------------

def _block_weights(W, bias=None, ones=False):
    """[10,10] weight + optional bias row -> [128,128] bf16: 12 diagonal
    10x10 blocks, bias broadcast from the ones-lane row, optional ones
    passthrough at [120,120]."""
    blk = np.zeros((128, 128), np.float32)
    for g in range(SLOTS):
        blk[D * g:D * g + D, D * g:D * g + D] = W
        if bias is not None:
            blk[ONES_P, D * g:D * g + D] = bias
    if ones:
        blk[ONES_P, ONES_P] = 1.0
    return blk.astype(BF)


def _prep_consts(A, W1, b1, W2, b2, W3, b3):
    A64 = np.asarray(A, np.float64)
    W164 = np.asarray(W1, np.float64)
    M1 = (A64.T @ W164[:D] + A64 @ W164[D:]).astype(np.float32)
    return {
        "BD1": _block_weights(M1, np.asarray(b1, np.float32), ones=True),
        "BD2": _block_weights(np.asarray(W2, np.float32),
                              np.asarray(b2, np.float32), ones=True),
        "BD3a": _block_weights(np.asarray(W3[:D], np.float32),
                               np.asarray(b3, np.float32), ones=False),
        "BD3b": _block_weights(np.asarray(W3[D:], np.float32), ones=False),
    }


# ---------------------------------------------------------------------------
# Bass program
# ---------------------------------------------------------------------------

def _build_program(split_waits=True, n_tiles=None):
    import concourse.bass as bass
    import concourse.mybir as mybir
    from concourse.tile import TileContext

    f32 = mybir.dt.float32
    bf16 = mybir.dt.bfloat16
    Relu = mybir.ActivationFunctionType.Relu
    H = C_TILE // 2  # 512, one PSUM bank per matmul

    nc = bass.Bass("TRN2", target_bir_lowering=False, debug=False)
    Xc = nc.dram_tensor("Xc", [PD, C_DEV], bf16, kind="ExternalInput")
    Zc = nc.dram_tensor("Zc", [PD, C_DEV], bf16, kind="ExternalOutput")
    CP = nc.dram_tensor("CPAD", [128 - PD, C_TILE], bf16, kind="ExternalInput")
    dws = {n: nc.dram_tensor(n, [128, 128], bf16, kind="ExternalInput")
           for n in ("BD1", "BD2", "BD3a", "BD3b")}

    xa, za = Xc.ap(), Zc.ap()
    T = N_TILES if n_tiles is None else n_tiles

    with TileContext(nc) as tc:
        with (
            tc.tile_pool(name="consts", bufs=1) as cpool,
            tc.tile_pool(name="xin", bufs=6) as xpool,
            tc.tile_pool(name="mid", bufs=3) as midpool,
            tc.tile_pool(name="zout", bufs=3) as zpool,
            tc.tile_pool(name="ps", bufs=4, space="PSUM") as pspool,
        ):
            sw = {}
            for n in ("BD1", "BD2", "BD3a", "BD3b"):
                t = cpool.tile([128, 128], bf16, tag=n)
                nc.sync.dma_start(out=t, in_=dws[n].ap())
                sw[n] = t

            # ones-lane / zero-pad partitions: written once per xin buffer,
            # loads only ever touch [0:PD]
            for _ in range(6):
                t = xpool.tile([128, C_TILE], bf16, tag="xin")
                nc.sync.dma_start(out=t[PD:128, :], in_=CP.ap())

            st = {}

            def s_load(it):
                xin = xpool.tile([128, C_TILE], bf16, tag="xin", name="xin")
                nc.sync.dma_start(
                    out=xin[0:PD, :], in_=xa[:, it * C_TILE:(it + 1) * C_TILE]
                )
                st[it] = {"xin": xin}

            def s_mm1(it):
                hps = pspool.tile([128, C_TILE], f32, tag="ps", name="hps")
                for j in (0, 1):
                    nc.tensor.matmul(
                        hps[:, H * j:H * (j + 1)], sw["BD1"],
                        st[it]["xin"][:, H * j:H * (j + 1)],
                        start=True, stop=True,
                    )
                hsb = midpool.tile([128, C_TILE], bf16, tag="hsb", name="hsb")
                nc.scalar.activation(hsb, hps, Relu)
                st[it]["hsb"] = hsb

            def s_mm2(it):
                hsb = st[it].pop("hsb")
                heps = pspool.tile([128, C_TILE], f32, tag="ps", name="heps")
                for j in (0, 1):
                    nc.tensor.matmul(
                        heps[:, H * j:H * (j + 1)], sw["BD2"],
                        hsb[:, H * j:H * (j + 1)], start=True, stop=True,
                    )
                hesb = midpool.tile([128, C_TILE], bf16, tag="hesb", name="hesb")
                nc.vector.tensor_scalar_max(hesb, heps, 0.0)
                st[it]["hesb"] = hesb

            def s_mm3(it):
                xin = st[it].pop("xin")
                hesb = st[it].pop("hesb")
                zps = pspool.tile([128, C_TILE], f32, tag="ps", name="zps")
                for j in (0, 1):
                    nc.tensor.matmul(
                        zps[:, H * j:H * (j + 1)], sw["BD3a"],
                        xin[:, H * j:H * (j + 1)], start=True, stop=False,
                    )
                for j in (0, 1):
                    nc.tensor.matmul(
                        zps[:, H * j:H * (j + 1)], sw["BD3b"],
                        hesb[:, H * j:H * (j + 1)], start=False, stop=True,
                    )
                zt = zpool.tile([128, C_TILE], bf16, tag="zt", name="zt")
                # alternate relu3 engine per tile to balance ACT/DVE load
                if it % 2 == 0:
                    nc.scalar.activation(zt, zps, Relu)
                else:
                    nc.vector.tensor_scalar_max(zt, zps, 0.0)
                st[it]["zt"] = zt

            def s_store(it):
                zt = st.pop(it)["zt"]
                nc.gpsimd.dma_start(
                    out=za[:, it * C_TILE:(it + 1) * C_TILE], in_=zt[0:PD, :]
                )

            # stage-offset software pipeline: each engine FIFO interleaves
            # across tiles, so tile t+1 matmuls never queue behind tile t
            # relus
            for r in range(T + 5):
                if r < T:
                    s_load(r)
                if 0 <= r - 2 < T:
                    s_mm1(r - 2)
                if 0 <= r - 3 < T:
                    s_mm2(r - 3)
                if 0 <= r - 4 < T:
                    s_mm3(r - 4)
                if 0 <= r - 5 < T:
                    s_store(r - 5)

    if split_waits:
        _split_sync_waits(nc, limit=1)
    return nc


_CACHED = {}


# ---------------------------------------------------------------------------
# Host-side pack / unpack
# ---------------------------------------------------------------------------

def _pack_inputs(X):
    """[B,10] f32 -> per-core [120, C_DEV] bf16, feature-major dense."""
    Xb = np.asarray(X, np.float32).astype(BF)
    Xp = np.zeros((N_CORES, R_CAP, D), BF)
    Xp[:, :ROWS_PER_CORE] = Xb.reshape(N_CORES, ROWS_PER_CORE, D)
    # [cores, C, slots, D] -> [cores, slots, D, C]
    Xt = Xp.reshape(N_CORES, C_DEV, SLOTS, D).transpose(0, 2, 3, 1)
    return [np.ascontiguousarray(Xt[c]).reshape(PD, C_DEV) for c in range(N_CORES)]


def _unpack_outputs(Zs):
    """per-core [120, C_DEV] bf16 -> [B,10] f32."""
    Z = np.stack(Zs).reshape(N_CORES, SLOTS, D, C_DEV)
    Z = Z.transpose(0, 3, 1, 2).reshape(N_CORES, R_CAP, D)[:, :ROWS_PER_CORE]
    return np.ascontiguousarray(Z.reshape(B_TOTAL, D)).astype(np.float32)


def kernel(X, A, W1, b1, W2, b2, W3, b3):
    _apply_drain_patch()
    _apply_verifier_patch()
    from concourse.bass_utils import run_bass_kernel_spmd

    consts = _prep_consts(A, W1, b1, W2, b2, W3, b3)

    if "nc" not in _CACHED:
        _CACHED["nc"] = _build_program()
    nc = _CACHED["nc"]

    cpad = np.zeros((128 - PD, C_TILE), BF)
    cpad[0] = 1.0  # ones-lane at partition PD
    xcores = _pack_inputs(X)
    in_maps = []
    for c in range(N_CORES):
        m = {"Xc": xcores[c], "CPAD": cpad}
        m.update(consts)
        in_maps.append(m)

    res = run_bass_kernel_spmd(nc, in_maps, core_ids=list(range(N_CORES)))
    _CACHED["last_results"] = res
    return _unpack_outputs([res.results[c]["Zc"] for c in range(N_CORES)])
